# revision 12
# baseline (speedup 1.0000x reference)
"""GCN message-passing kernel for 8 Trainium2 NeuronCores.

Strategy: shard CHANNELS across the 8 cores (C=1280 -> 160 ch/core). Each core
computes the full output for its channel slice with zero collectives:
  - edge-encoder MLP: gamma/beta only for this core's 160 channels (W2 column
    shard), with the full h = relu(pose @ W1 + b1) recomputed per core on PE
    using 2x row-tiling (K=9 fits a 32-row group; two k-tiles run concurrently).
  - edges are host-sorted by dst and padded per 128-node dst window; the
    scatter-sum is a one-hot matmul on PE accumulating into PSUM per window.
    Blocks are processed in PAIRS with fp8 DoubleRow (K=256) so the one-hot
    weight load amortizes over 6 matmuls and the m stream halves.
  - the per-edge gather image[src] is an indirect DMA from an HBM-resident
    per-core image slice laid out [node, hw, ch] in bf16.
  - mean = PSUM evacuation with per-partition scale 1/cnt (0 for empty nodes).
"""

import sys

sys.path.insert(0, "/opt/trn_rl_repo")

import numpy as np
import ml_dtypes

import concourse.bass as bass
import concourse.mybir as mybir
from concourse.tile import TileContext
from concourse.bass_utils import run_bass_kernel_spmd

BF16 = ml_dtypes.bfloat16
FP8 = ml_dtypes.float8_e4m3
P = 128
N_CORES = 8
CH_EDGES = 1024  # edges per h-chunk
BPC = CH_EDGES // P  # blocks per chunk

ET_MODE = "nodr"  # eT matmul: "nodr" (fp8+FWL) | "dr" (DoubleRow)
SC_MODE = "dr"  # scatter: "dr" (fp8 DoubleRow pairs) | "bf16"


def _split_excess_waits(nc):
    """This walrus build only encodes 1 sem-wait per instruction; hoist extra
    waits onto same-engine NoOps placed just before (engines run in order)."""
    for bb in nc.main_func.blocks:
        new_insts = []
        for ins in bb.instructions:
            si = ins.sync_info
            limit = 1
            if si is not None and si.on_wait and len(si.on_wait) > limit:
                waits = list(si.on_wait)
                extra, keep = waits[:-limit], waits[-limit:]
                for k, w in enumerate(extra):
                    nop = mybir.InstNoOp(name=f"{ins.name}-ws-{k}", ins=[], outs=[])
                    nop.engine = ins.engine
                    nop.sync_info = mybir.SyncInfo(on_wait=[w], on_update=[])
                    new_insts.append(nop)
                si.on_wait = keep
            new_insts.append(ins)
        bb.instructions[:] = new_insts


def _host_prep(pose, image, W1, b1, W2, b2, src, dst):
    """Sort/pad edges by dst window, build per-core shards and onehot pairs."""
    E = pose.shape[0]
    Nn, C, H, Wsp = image.shape
    HW = H * Wsp
    CS = C // N_CORES
    F = CS * HW
    n_win = Nn // P

    src = np.asarray(src).astype(np.int64)
    dst = np.asarray(dst).astype(np.int64)

    order = np.argsort(dst, kind="stable")
    blk_edge = []  # [B, 128] edge id, -1 = pad
    blk_win = []
    for w in range(n_win):
        sel = order[(dst[order] >= w * P) & (dst[order] < (w + 1) * P)]
        nb = max(1, -(-len(sel) // P))
        if nb % 2:  # even block count per window -> clean DR pairs
            nb += 1
        for b in range(nb):
            seg = sel[b * P : (b + 1) * P]
            row = np.full(P, -1, np.int64)
            row[: len(seg)] = seg
            blk_edge.append(row)
            blk_win.append(w)
    blk_edge = np.stack(blk_edge)  # [B, 128]
    B = len(blk_edge)

    valid = blk_edge >= 0
    eids = np.where(valid, blk_edge, 0)

    # gather src per edge slot (pad -> node 0)
    blk_src = np.where(valid, src[eids], 0).astype(np.int32)  # [B,128]
    # onehot: [B, 128 edge, 128 local-node], zero row for pads
    loc = (np.where(valid, dst[eids], 0) - np.asarray(blk_win)[:, None] * P).astype(
        np.int64
    )
    oh = np.zeros((B, P, P), np.float32)
    bi, pi = np.nonzero(valid)
    oh[bi, pi, loc[bi, pi]] = 1.0

    # pairs (within window; B even per window by construction)
    pair_blk = []  # [NP, 2]
    pair_win = []
    b = 0
    while b < B:
        assert blk_win[b] == blk_win[b + 1]
        pair_blk.append((b, b + 1))
        pair_win.append(blk_win[b])
        b += 2
    NP = len(pair_blk)
    first_pair = {}
    last_pair = {}
    for pi_, w in enumerate(pair_win):
        first_pair.setdefault(w, pi_)
        last_pair[w] = pi_

    # edge chunks for h: pad edge count to CH_EDGES multiple
    Ep = -(-B * P // CH_EDGES) * CH_EDGES
    NCH = Ep // CH_EDGES

    # poseT padded: [9, Ep]
    pose_pad = np.zeros((Ep, 9), np.float32)
    pose_pad[: B * P] = np.where(valid.reshape(-1, 1), pose[eids.reshape(-1)], 0.0)
    poseT = np.ascontiguousarray(pose_pad.T.astype(BF16))  # [9, Ep]

    # oh, laid out for the scatter mode
    if SC_MODE == "dr":
        # per pair: [128 slot, 2*128]: [:, j*128+n] = oh[blk_j][slot, n], fp8
        oh_pairs = np.zeros((P, NP * 2 * P), np.float32)
        for pi_, (ba, bb) in enumerate(pair_blk):
            oh_pairs[:, pi_ * 2 * P : pi_ * 2 * P + P] = oh[ba]
            oh_pairs[:, pi_ * 2 * P + P : (pi_ + 1) * 2 * P] = oh[bb]
        oh_host = oh_pairs.astype(FP8)  # [128, NP*256]
    else:
        oh_host = (
            oh.transpose(1, 0, 2).reshape(P, B * P).astype(BF16)
        )  # [128, B*128]

    b2_allzero = not np.any(b2)
    cnt = np.bincount(dst, minlength=Nn).astype(np.float32)
    recip = np.where(cnt > 0, 1.0 / np.maximum(cnt, 1.0), 0.0).astype(np.float32)
    recip_t = np.ascontiguousarray(recip.reshape(n_win, P).T)  # [P, n_win]

    KT = C // P
    b1t = np.ascontiguousarray(b1.astype(np.float32).reshape(KT, P).T)  # [P, KT]
    idx_t = np.ascontiguousarray(blk_src.T)  # [P, B]

    shared = dict(
        poseT=poseT,
        w1=W1.astype(BF16),
        b1t=b1t,
        idx=idx_t,
        oh=oh_host,
        recip=recip_t,
    )
    in_maps = []
    for j in range(N_CORES):
        c0 = j * CS
        cols_g = [2 * (c0 + i) for i in range(CS)]
        cols_b = [2 * (c0 + i) + 1 for i in range(CS)]
        cols = cols_g + cols_b
        w2f8 = W2[:, cols].astype(FP8)  # [C, 2*CS]
        b2row = b2[cols].reshape(1, -1).astype(BF16)  # [1, 2*CS]
        img = (
            image[:, c0 : c0 + CS]
            .transpose(0, 2, 3, 1)
            .reshape(Nn, F)
            .astype(BF16)
        )  # [Nn, F] layout [n, hw, c]
        in_maps.append(dict(shared, w2f8=w2f8, b2row=b2row, image=img))

    meta = dict(
        E=E, Nn=Nn, C=C, HW=HW, CS=CS, F=F, n_win=n_win, B=B, Ep=Ep, KT=KT,
        NP=NP, NCH=NCH, pair_blk=pair_blk, pair_win=pair_win,
        first_pair=first_pair, last_pair=last_pair, b2_allzero=b2_allzero,
    )
    return in_maps, meta


def _build(meta):
    Nn, CS, F, HW = meta["Nn"], meta["CS"], meta["F"], meta["HW"]
    n_win, B, Ep, KT = meta["n_win"], meta["B"], meta["Ep"], meta["KT"]
    C, NP, NCH = meta["C"], meta["NP"], meta["NCH"]
    pair_blk, pair_win = meta["pair_blk"], meta["pair_win"]
    first_pair, last_pair = meta["first_pair"], meta["last_pair"]
    f32 = mybir.dt.float32
    bf16 = mybir.dt.bfloat16
    fp8 = mybir.dt.float8e4
    i32 = mybir.dt.int32
    FS = F + CS  # scatter width: gamma*x features + beta column block
    seg_cols = [(s, min(512, FS - s)) for s in range(0, FS, 512)]
    m_dt = fp8 if SC_MODE == "dr" else bf16
    oh_dt = fp8 if SC_MODE == "dr" else bf16
    oh_w = NP * 2 * P if SC_MODE == "dr" else B * P

    nc = bass.Bass()
    poseT_d = nc.declare_dram_parameter("poseT", [9, Ep], bf16, isOutput=False)
    w1_d = nc.declare_dram_parameter("w1", [9, C], bf16, isOutput=False)
    b1t_d = nc.declare_dram_parameter("b1t", [P, KT], f32, isOutput=False)
    w2f8_d = nc.declare_dram_parameter("w2f8", [C, 2 * CS], fp8, isOutput=False)
    b2_d = nc.declare_dram_parameter("b2row", [1, 2 * CS], bf16, isOutput=False)
    img_d = nc.declare_dram_parameter("image", [Nn, F], bf16, isOutput=False)
    idx_d = nc.declare_dram_parameter("idx", [P, B], i32, isOutput=False)
    oh_d = nc.declare_dram_parameter("oh", [P, oh_w], oh_dt, isOutput=False)
    recip_d = nc.declare_dram_parameter("recip", [P, n_win], f32, isOutput=False)
    out_d = nc.declare_dram_parameter("out", [Nn, F], bf16, isOutput=True)

    with TileContext(nc) as tc:
        with (
            tc.tile_pool(name="const", bufs=1) as constp,
            tc.tile_pool(name="ht", bufs=3) as htp,
            tc.tile_pool(name="gb", bufs=6) as gbp,
            tc.tile_pool(name="xg", bufs=6) as xp,
            tc.tile_pool(name="mm", bufs=4) as mp,
            tc.tile_pool(name="outp", bufs=2) as outp,
            tc.tile_pool(name="pw", bufs=1, space="PSUM") as pwp,
            tc.tile_pool(name="ps", bufs=2, space="PSUM") as psp,
        ):
            # ---- preload constants (first chunk's pose slice first, so the
            # PE can start immediately; the rest streams behind it) ----
            w1_sb = constp.tile([41, C], bf16)
            nc.sync.dma_start(out=w1_sb[0:9, :], in_=w1_d[:])
            nc.sync.dma_start(out=w1_sb[32:41, :], in_=w1_d[:])
            poseT_sb = constp.tile([41, Ep], bf16)
            nc.scalar.dma_start(
                out=poseT_sb[0:9, :CH_EDGES], in_=poseT_d[:, :CH_EDGES]
            )
            nc.scalar.dma_start(
                out=poseT_sb[32:41, :CH_EDGES], in_=poseT_d[:, :CH_EDGES]
            )
            nc.sync.dma_start(out=poseT_sb[0:9, CH_EDGES:], in_=poseT_d[:, CH_EDGES:])
            nc.sync.dma_start(
                out=poseT_sb[32:41, CH_EDGES:], in_=poseT_d[:, CH_EDGES:]
            )
            b1_sb = constp.tile([P, KT], f32)
            nc.sync.dma_start(out=b1_sb[:], in_=b1t_d[:])
            w2_sb = constp.tile([P, KT * 2 * CS], fp8)
            for t in range(KT):
                nc.sync.dma_start(
                    out=w2_sb[:, t * 2 * CS : (t + 1) * 2 * CS],
                    in_=w2f8_d[t * P : (t + 1) * P, :],
                )
            b2_sb = constp.tile([1, 2 * CS], bf16)
            if not meta["b2_allzero"]:
                nc.sync.dma_start(out=b2_sb[:], in_=b2_d[:])
            idx_sb = constp.tile([P, B], i32)
            nc.sync.dma_start(out=idx_sb[:], in_=idx_d[:])
            oh_sb = constp.tile([P, oh_w], oh_dt)
            nc.sync.dma_start(out=oh_sb[:], in_=oh_d[:])
            recip_sb = constp.tile([P, n_win], f32)
            nc.sync.dma_start(out=recip_sb[:], in_=recip_d[:])
            ones_sb = constp.tile([1, P], bf16)
            if not meta["b2_allzero"]:
                nc.gpsimd.memset(ones_sb[:], 1.0)

            w23 = w2_sb.rearrange("p (t c) -> p t c", t=KT)

            hT_tiles = {}  # chunk -> tile
            h_queue = []  # pending (ci, g, e0) h iterations, FIFO
            h_queued_ci = -1  # last chunk whose iterations were enqueued
            evac_ctr = [0]

            def queue_h(ci):
                nonlocal h_queued_ci
                while h_queued_ci < ci:
                    h_queued_ci += 1
                    if h_queued_ci >= NCH:
                        return
                    hT_tiles[h_queued_ci] = htp.tile(
                        [P, KT * CH_EDGES], fp8, tag="ht", name=f"hT{h_queued_ci}"
                    )
                    for g in range(KT // 2):
                        for e0 in range(0, CH_EDGES, 512):
                            h_queue.append((h_queued_ci, g, e0))

            def emit_h_iter(ci, g, e0):
                """one 2x row-tiled K=9 matmul pair of the hT chunk:
                hT = relu(W1.T posT + b1), stored fp8."""
                hT = hT_tiles[ci]
                e_lo = ci * CH_EDGES
                pha = psp.tile([P, 512], f32, tag="ps")
                phb = psp.tile([P, 512], f32, tag="ps")
                t0, t1 = 2 * g, 2 * g + 1
                nc.tensor.matmul(
                    out=pha[:],
                    lhsT=w1_sb[0:9, t0 * P : (t0 + 1) * P],
                    rhs=poseT_sb[0:9, e_lo + e0 : e_lo + e0 + 512],
                    start=True,
                    stop=True,
                    tile_position=(0, 0),
                )
                nc.tensor.matmul(
                    out=phb[:],
                    lhsT=w1_sb[32:41, t1 * P : (t1 + 1) * P],
                    rhs=poseT_sb[32:41, e_lo + e0 : e_lo + e0 + 512],
                    start=True,
                    stop=True,
                    tile_position=(32, 0),
                )
                for t, ph in ((t0, pha), (t1, phb)):
                    dstv = hT[:, t * CH_EDGES + e0 : t * CH_EDGES + e0 + 512]
                    # alternate PSUM evacuation between ACT and DVE
                    evac_ctr[0] += 1
                    if evac_ctr[0] % 3 != 2:
                        nc.scalar.activation(
                            dstv,
                            ph[:],
                            mybir.ActivationFunctionType.Relu,
                            bias=b1_sb[:, t : t + 1],
                            scale=1.0,
                        )
                    else:
                        nc.vector.tensor_scalar(
                            out=dstv,
                            in0=ph[:],
                            scalar1=b1_sb[:, t : t + 1],
                            scalar2=0.0,
                            op0=mybir.AluOpType.add,
                            op1=mybir.AluOpType.max,
                        )

            def pump_h(n):
                for _ in range(min(n, len(h_queue))):
                    emit_h_iter(*h_queue.pop(0))

            def flush_h(ci):
                """emit all pending h work for chunks <= ci"""
                queue_h(ci)
                while h_queue and h_queue[0][0] <= ci:
                    emit_h_iter(*h_queue.pop(0))

            def emit_gb(b, hT, ci):
                """gamma/beta for one 128-edge block -> bf16 [128, 2CS] half."""
                bi = b - ci * BPC
                pe_ps = psp.tile([P, 512], f32, tag="ps")
                if ET_MODE == "dr":
                    hT3 = hT.rearrange("p (t e) -> p t e", t=KT)
                    for t2 in range(KT // 2):
                        nc.tensor.matmul(
                            out=pe_ps[:, : 2 * CS],
                            lhsT=hT3[:, 2 * t2 : 2 * t2 + 2, bi * P : (bi + 1) * P],
                            rhs=w23[:, 2 * t2 : 2 * t2 + 2, :],
                            start=(t2 == 0),
                            stop=(t2 == KT // 2 - 1 and meta["b2_allzero"]),
                            perf_mode=mybir.MatmulPerfMode.DoubleRow,
                        )
                else:  # nodr: plain fp8, FWL weight loads
                    for t in range(KT):
                        nc.tensor.matmul(
                            out=pe_ps[:, : 2 * CS],
                            lhsT=hT[
                                :, t * CH_EDGES + bi * P : t * CH_EDGES + (bi + 1) * P
                            ],
                            rhs=w23[:, t, :],
                            start=(t == 0),
                            stop=(t == KT - 1 and meta["b2_allzero"]),
                        )
                if not meta["b2_allzero"]:
                    nc.tensor.matmul(
                        out=pe_ps[:, : 2 * CS],
                        lhsT=ones_sb[:1, :P],
                        rhs=b2_sb[:1, :],
                        start=False,
                        stop=True,
                    )
                return pe_ps

            # ---- main pipeline over pairs ----
            psw = None

            for pi_ in range(NP):
                ba, bb = pair_blk[pi_]
                w = pair_win[pi_]
                ci_need = bb // BPC
                flush_h(ci_need)  # h this pair depends on: emit now
                queue_h(ci_need + 1)  # next chunk's h: trickle between pairs

                # ---- gather X for the pair: [128, 2F]
                X2 = xp.tile([P, 2 * F], bf16, tag="xg")
                for j, b in enumerate((ba, bb)):
                    nc.gpsimd.indirect_dma_start(
                        out=X2[:, j * F : (j + 1) * F],
                        out_offset=None,
                        in_=img_d[:],
                        in_offset=bass.IndirectOffsetOnAxis(
                            ap=idx_sb[:, b : b + 1], axis=0
                        ),
                    )

                # ---- gamma (bf16, feeds DVE) and beta (m_dt, direct into m2's
                # 6th scatter segment) for both blocks
                gb2 = gbp.tile([P, 2 * CS], bf16, tag="gb")
                m2 = mp.tile([P, 2 * FS], m_dt, tag="mm")
                for j, b in enumerate((ba, bb)):
                    pe_ps = emit_gb(b, hT_tiles[b // BPC], b // BPC)
                    nc.scalar.activation(
                        gb2[:, j * CS : (j + 1) * CS],
                        pe_ps[:, :CS],
                        mybir.ActivationFunctionType.Sigmoid,
                    )
                    nc.scalar.activation(
                        m2[:, j * FS + F : (j + 1) * FS],
                        pe_ps[:, CS : 2 * CS],
                        mybir.ActivationFunctionType.Sigmoid,
                    )
                    pump_h(1)  # keep PE fed while sigmoid evacuates pe_ps

                # ---- m = gamma (bcast over hw) * X, per block
                for j in range(2):
                    g_b = (
                        gb2[:, j * CS : (j + 1) * CS]
                        .rearrange("p (o c) -> p o c", o=1)
                        .to_broadcast([P, HW, CS])
                    )
                    nc.vector.tensor_tensor(
                        out=m2[:, j * FS : j * FS + F].rearrange(
                            "p (o c) -> p o c", o=HW
                        ),
                        in0=X2[:, j * F : (j + 1) * F].rearrange(
                            "p (o c) -> p o c", o=HW
                        ),
                        in1=g_b,
                        op=mybir.AluOpType.mult,
                    )

                # ---- scatter matmuls into the window PSUM
                first = first_pair[w] == pi_
                last = last_pair[w] == pi_
                if first:
                    psw = pwp.tile([P, FS], f32, tag="pw")
                psw_l = psw

                if SC_MODE == "dr":
                    oh2 = oh_sb[:, pi_ * 2 * P : (pi_ + 1) * 2 * P].rearrange(
                        "p (j n) -> p j n", j=2
                    )
                    m3 = m2.rearrange("p (j f) -> p j f", j=2)
                    for s, width in seg_cols:
                        nc.tensor.matmul(
                            out=psw_l[:, s : s + width],
                            lhsT=oh2,
                            rhs=m3[:, :, s : s + width],
                            start=first,
                            stop=last,
                            perf_mode=mybir.MatmulPerfMode.DoubleRow,
                            skip_group_check=True,
                        )
                else:
                    for j, b in enumerate((ba, bb)):
                        oht = oh_sb[:, b * P : (b + 1) * P]
                        for s, width in seg_cols:
                            nc.tensor.matmul(
                                out=psw_l[:, s : s + width],
                                lhsT=oht,
                                rhs=m2[:, j * FS + s : j * FS + s + width],
                                start=first and j == 0,
                                stop=last and j == 1,
                                skip_group_check=True,
                            )

                pump_h(1)

                if last:
                    # ---- evacuate window: out = psw*recip + (beta_seg*recip)
                    # split by column halves across ACT and DVE so the PSUM
                    # window frees ~2x sooner (it gates the next window)
                    bs = outp.tile([P, CS], bf16, tag="bs")
                    nc.scalar.activation(
                        bs[:],
                        psw_l[:, F:FS],
                        mybir.ActivationFunctionType.Copy,
                        scale=recip_sb[:, w : w + 1],
                    )
                    HF = F // 2
                    HO = HW // 2
                    bs_b = bs.rearrange("p (o c) -> p o c", o=1)
                    of = outp.tile([P, F], bf16, tag="of")
                    of3 = of.rearrange("p (o c) -> p o c", o=HW)
                    psw3 = psw_l[:, :F].rearrange("p (o c) -> p o c", o=HW)
                    om = outp.tile([P, HF], bf16, tag="om")
                    nc.scalar.activation(
                        om[:],
                        psw_l[:, :HF],
                        mybir.ActivationFunctionType.Copy,
                        scale=recip_sb[:, w : w + 1],
                    )
                    nc.vector.scalar_tensor_tensor(
                        out=of3[:, HO:, :],
                        in0=psw3[:, HO:, :],
                        scalar=recip_sb[:, w : w + 1],
                        in1=bs_b.to_broadcast([P, HO, CS]),
                        op0=mybir.AluOpType.mult,
                        op1=mybir.AluOpType.add,
                    )
                    nc.vector.tensor_tensor(
                        out=of3[:, :HO, :],
                        in0=om.rearrange("p (o c) -> p o c", o=HO),
                        in1=bs_b.to_broadcast([P, HO, CS]),
                        op=mybir.AluOpType.add,
                    )
                    nc.sync.dma_start(out=out_d[w * P : (w + 1) * P, :], in_=of[:])

    _split_excess_waits(nc)
    return nc


def _run(inputs, trace=False, trace_kwargs=None):
    pose = np.asarray(inputs["pose"], np.float32)
    image = np.asarray(inputs["image"], np.float32)
    W1 = np.asarray(inputs["W1"], np.float32)
    b1 = np.asarray(inputs["b1"], np.float32)
    W2 = np.asarray(inputs["W2"], np.float32)
    b2 = np.asarray(inputs["b2"], np.float32)
    src = np.asarray(inputs["src"])
    dst = np.asarray(inputs["dst"])

    in_maps, meta = _host_prep(pose, image, W1, b1, W2, b2, src, dst)
    nc = _build(meta)
    kw = {}
    if trace:
        kw = dict(trace=True, trace_kwargs=trace_kwargs or {})
    res = run_bass_kernel_spmd(nc, in_maps, core_ids=list(range(N_CORES)), **kw)
    Nn, C, HW, CS = meta["Nn"], meta["C"], meta["HW"], meta["CS"]
    H = int(np.sqrt(HW))
    out = np.empty((Nn, C, H, HW // H), np.float32)
    for j in range(N_CORES):
        oc = np.asarray(res.results[j]["out"]).astype(np.float32)
        out[:, j * CS : (j + 1) * CS] = (
            oc.reshape(Nn, HW, CS).transpose(0, 2, 1).reshape(Nn, CS, H, HW // H)
        )
    return out, res


def kernel(**inputs) -> np.ndarray:
    out, _ = _run(inputs)
    return out


# revision 13
# speedup vs baseline: 1.1835x; 1.1835x over previous
"""GCN message-passing kernel for 8 Trainium2 NeuronCores.

Strategy: shard CHANNELS across the 8 cores (C=1280 -> 160 ch/core). Each core
computes the full output for its channel slice with zero collectives:
  - edge-encoder MLP: gamma/beta only for this core's 160 channels (W2 column
    shard), with the full h = relu(pose @ W1 + b1) recomputed per core on PE
    using 2x row-tiling (K=9 fits a 32-row group; two k-tiles run concurrently).
  - edges are host-sorted by dst and padded per 128-node dst window; the
    scatter-sum is a one-hot matmul on PE accumulating into PSUM per window.
    Blocks are processed in PAIRS with fp8 DoubleRow (K=256) so the one-hot
    weight load amortizes over 6 matmuls and the m stream halves.
  - the per-edge gather image[src] is an indirect DMA from an HBM-resident
    per-core image slice laid out [node, hw, ch] in bf16.
  - mean = PSUM evacuation with per-partition scale 1/cnt (0 for empty nodes).
"""

import sys

sys.path.insert(0, "/opt/trn_rl_repo")

import numpy as np
import ml_dtypes

import concourse.bass as bass
import concourse.mybir as mybir
from concourse.tile import TileContext
from concourse.bass_utils import run_bass_kernel_spmd

BF16 = ml_dtypes.bfloat16
FP8 = ml_dtypes.float8_e4m3
P = 128
N_CORES = 8
CH_EDGES = 1024  # edges per h-chunk
BPC = CH_EDGES // P  # blocks per chunk

ET_MODE = "nodr"  # eT matmul: "nodr" (fp8+FWL) | "dr" (DoubleRow)
SC_MODE = "bf16"  # scatter: "dr" (fp8 DoubleRow pairs) | "bf16"


def _split_excess_waits(nc):
    """This walrus build only encodes 1 sem-wait per instruction; hoist extra
    waits onto same-engine NoOps placed just before (engines run in order)."""
    for bb in nc.main_func.blocks:
        new_insts = []
        for ins in bb.instructions:
            si = ins.sync_info
            limit = 1
            if si is not None and si.on_wait and len(si.on_wait) > limit:
                waits = list(si.on_wait)
                extra, keep = waits[:-limit], waits[-limit:]
                for k, w in enumerate(extra):
                    nop = mybir.InstNoOp(name=f"{ins.name}-ws-{k}", ins=[], outs=[])
                    nop.engine = ins.engine
                    nop.sync_info = mybir.SyncInfo(on_wait=[w], on_update=[])
                    new_insts.append(nop)
                si.on_wait = keep
            new_insts.append(ins)
        bb.instructions[:] = new_insts


def _host_prep(pose, image, W1, b1, W2, b2, src, dst):
    """Sort/pad edges by dst window, build per-core shards and onehot pairs."""
    E = pose.shape[0]
    Nn, C, H, Wsp = image.shape
    HW = H * Wsp
    CS = C // N_CORES
    F = CS * HW
    n_win = Nn // P

    src = np.asarray(src).astype(np.int64)
    dst = np.asarray(dst).astype(np.int64)

    order = np.argsort(dst, kind="stable")
    blk_edge = []  # [B, 128] edge id, -1 = pad
    blk_win = []
    for w in range(n_win):
        sel = order[(dst[order] >= w * P) & (dst[order] < (w + 1) * P)]
        nb = max(1, -(-len(sel) // P))
        if nb % 2:  # even block count per window -> clean DR pairs
            nb += 1
        for b in range(nb):
            seg = sel[b * P : (b + 1) * P]
            row = np.full(P, -1, np.int64)
            row[: len(seg)] = seg
            blk_edge.append(row)
            blk_win.append(w)
    blk_edge = np.stack(blk_edge)  # [B, 128]
    B = len(blk_edge)

    valid = blk_edge >= 0
    eids = np.where(valid, blk_edge, 0)

    # gather src per edge slot (pad -> node 0)
    blk_src = np.where(valid, src[eids], 0).astype(np.int32)  # [B,128]
    # onehot: [B, 128 edge, 128 local-node], zero row for pads
    loc = (np.where(valid, dst[eids], 0) - np.asarray(blk_win)[:, None] * P).astype(
        np.int64
    )
    oh = np.zeros((B, P, P), np.float32)
    bi, pi = np.nonzero(valid)
    oh[bi, pi, loc[bi, pi]] = 1.0

    # pairs (within window; B even per window by construction)
    pair_blk = []  # [NP, 2]
    pair_win = []
    b = 0
    while b < B:
        assert blk_win[b] == blk_win[b + 1]
        pair_blk.append((b, b + 1))
        pair_win.append(blk_win[b])
        b += 2
    NP = len(pair_blk)
    first_pair = {}
    last_pair = {}
    for pi_, w in enumerate(pair_win):
        first_pair.setdefault(w, pi_)
        last_pair[w] = pi_

    # edge chunks for h: pad edge count to CH_EDGES multiple
    Ep = -(-B * P // CH_EDGES) * CH_EDGES
    NCH = Ep // CH_EDGES

    # poseT padded: [9, Ep]
    pose_pad = np.zeros((Ep, 9), np.float32)
    pose_pad[: B * P] = np.where(valid.reshape(-1, 1), pose[eids.reshape(-1)], 0.0)
    poseT = np.ascontiguousarray(pose_pad.T.astype(BF16))  # [9, Ep]

    # oh, laid out for the scatter mode
    if SC_MODE == "dr":
        # per pair: [128 slot, 2*128]: [:, j*128+n] = oh[blk_j][slot, n], fp8
        oh_pairs = np.zeros((P, NP * 2 * P), np.float32)
        for pi_, (ba, bb) in enumerate(pair_blk):
            oh_pairs[:, pi_ * 2 * P : pi_ * 2 * P + P] = oh[ba]
            oh_pairs[:, pi_ * 2 * P + P : (pi_ + 1) * 2 * P] = oh[bb]
        oh_host = oh_pairs.astype(FP8)  # [128, NP*256]
    else:
        oh_host = (
            oh.transpose(1, 0, 2).reshape(P, B * P).astype(BF16)
        )  # [128, B*128]

    b2_allzero = not np.any(b2)
    cnt = np.bincount(dst, minlength=Nn).astype(np.float32)
    recip = np.where(cnt > 0, 1.0 / np.maximum(cnt, 1.0), 0.0).astype(np.float32)
    recip_t = np.ascontiguousarray(recip.reshape(n_win, P).T)  # [P, n_win]

    KT = C // P
    b1t = np.ascontiguousarray(b1.astype(np.float32).reshape(KT, P).T)  # [P, KT]
    idx_t = np.ascontiguousarray(blk_src.T)  # [P, B]

    shared = dict(
        poseT=poseT,
        w1=W1.astype(BF16),
        b1t=b1t,
        idx=idx_t,
        oh=oh_host,
        recip=recip_t,
    )
    in_maps = []
    for j in range(N_CORES):
        c0 = j * CS
        cols_g = [2 * (c0 + i) for i in range(CS)]
        cols_b = [2 * (c0 + i) + 1 for i in range(CS)]
        cols = cols_g + cols_b
        w2f8 = W2[:, cols].astype(FP8)  # [C, 2*CS]
        b2row = b2[cols].reshape(1, -1).astype(BF16)  # [1, 2*CS]
        img = (
            image[:, c0 : c0 + CS]
            .transpose(0, 2, 3, 1)
            .reshape(Nn, F)
            .astype(BF16)
        )  # [Nn, F] layout [n, hw, c]
        in_maps.append(dict(shared, w2f8=w2f8, b2row=b2row, image=img))

    meta = dict(
        E=E, Nn=Nn, C=C, HW=HW, CS=CS, F=F, n_win=n_win, B=B, Ep=Ep, KT=KT,
        NP=NP, NCH=NCH, pair_blk=pair_blk, pair_win=pair_win,
        first_pair=first_pair, last_pair=last_pair, b2_allzero=b2_allzero,
    )
    return in_maps, meta


def _build(meta):
    Nn, CS, F, HW = meta["Nn"], meta["CS"], meta["F"], meta["HW"]
    n_win, B, Ep, KT = meta["n_win"], meta["B"], meta["Ep"], meta["KT"]
    C, NP, NCH = meta["C"], meta["NP"], meta["NCH"]
    pair_blk, pair_win = meta["pair_blk"], meta["pair_win"]
    first_pair, last_pair = meta["first_pair"], meta["last_pair"]
    f32 = mybir.dt.float32
    bf16 = mybir.dt.bfloat16
    fp8 = mybir.dt.float8e4
    i32 = mybir.dt.int32
    FS = F + CS  # scatter width: gamma*x features + beta column block
    seg_cols = [(s, min(512, FS - s)) for s in range(0, FS, 512)]
    m_dt = fp8 if SC_MODE == "dr" else bf16
    oh_dt = fp8 if SC_MODE == "dr" else bf16
    oh_w = NP * 2 * P if SC_MODE == "dr" else B * P

    nc = bass.Bass()
    poseT_d = nc.declare_dram_parameter("poseT", [9, Ep], bf16, isOutput=False)
    w1_d = nc.declare_dram_parameter("w1", [9, C], bf16, isOutput=False)
    b1t_d = nc.declare_dram_parameter("b1t", [P, KT], f32, isOutput=False)
    w2f8_d = nc.declare_dram_parameter("w2f8", [C, 2 * CS], fp8, isOutput=False)
    b2_d = nc.declare_dram_parameter("b2row", [1, 2 * CS], bf16, isOutput=False)
    img_d = nc.declare_dram_parameter("image", [Nn, F], bf16, isOutput=False)
    idx_d = nc.declare_dram_parameter("idx", [P, B], i32, isOutput=False)
    oh_d = nc.declare_dram_parameter("oh", [P, oh_w], oh_dt, isOutput=False)
    recip_d = nc.declare_dram_parameter("recip", [P, n_win], f32, isOutput=False)
    out_d = nc.declare_dram_parameter("out", [Nn, F], bf16, isOutput=True)

    with TileContext(nc) as tc:
        with (
            tc.tile_pool(name="const", bufs=1) as constp,
            tc.tile_pool(name="ht", bufs=3) as htp,
            tc.tile_pool(name="gb", bufs=6) as gbp,
            tc.tile_pool(name="xg", bufs=6) as xp,
            tc.tile_pool(name="mm", bufs=4) as mp,
            tc.tile_pool(name="outp", bufs=2) as outp,
            tc.tile_pool(name="pw", bufs=1, space="PSUM") as pwp,
            tc.tile_pool(name="ps", bufs=2, space="PSUM") as psp,
        ):
            # ---- preload constants (first chunk's pose slice first, so the
            # PE can start immediately; the rest streams behind it) ----
            w1_sb = constp.tile([41, C], bf16)
            nc.sync.dma_start(out=w1_sb[0:9, :], in_=w1_d[:])
            nc.sync.dma_start(out=w1_sb[32:41, :], in_=w1_d[:])
            poseT_sb = constp.tile([41, Ep], bf16)
            nc.scalar.dma_start(
                out=poseT_sb[0:9, :CH_EDGES], in_=poseT_d[:, :CH_EDGES]
            )
            nc.scalar.dma_start(
                out=poseT_sb[32:41, :CH_EDGES], in_=poseT_d[:, :CH_EDGES]
            )
            nc.sync.dma_start(out=poseT_sb[0:9, CH_EDGES:], in_=poseT_d[:, CH_EDGES:])
            nc.sync.dma_start(
                out=poseT_sb[32:41, CH_EDGES:], in_=poseT_d[:, CH_EDGES:]
            )
            b1_sb = constp.tile([P, KT], f32)
            nc.sync.dma_start(out=b1_sb[:], in_=b1t_d[:])
            w2_sb = constp.tile([P, KT * 2 * CS], fp8)
            for t in range(KT):
                nc.sync.dma_start(
                    out=w2_sb[:, t * 2 * CS : (t + 1) * 2 * CS],
                    in_=w2f8_d[t * P : (t + 1) * P, :],
                )
            b2_sb = constp.tile([1, 2 * CS], bf16)
            if not meta["b2_allzero"]:
                nc.sync.dma_start(out=b2_sb[:], in_=b2_d[:])
            idx_sb = constp.tile([P, B], i32)
            nc.sync.dma_start(out=idx_sb[:], in_=idx_d[:])
            oh_sb = constp.tile([P, oh_w], oh_dt)
            nc.sync.dma_start(out=oh_sb[:], in_=oh_d[:])
            recip_sb = constp.tile([P, n_win], f32)
            nc.sync.dma_start(out=recip_sb[:], in_=recip_d[:])
            ones_sb = constp.tile([1, P], bf16)
            if not meta["b2_allzero"]:
                nc.gpsimd.memset(ones_sb[:], 1.0)

            w23 = w2_sb.rearrange("p (t c) -> p t c", t=KT)

            hT_tiles = {}  # chunk -> tile
            h_queue = []  # pending (ci, g, e0) h iterations, FIFO
            h_queued_ci = -1  # last chunk whose iterations were enqueued
            evac_ctr = [0]

            def queue_h(ci):
                nonlocal h_queued_ci
                while h_queued_ci < ci:
                    h_queued_ci += 1
                    if h_queued_ci >= NCH:
                        return
                    hT_tiles[h_queued_ci] = htp.tile(
                        [P, KT * CH_EDGES], fp8, tag="ht", name=f"hT{h_queued_ci}"
                    )
                    for g in range(KT // 2):
                        for e0 in range(0, CH_EDGES, 512):
                            h_queue.append((h_queued_ci, g, e0))

            def emit_h_iter(ci, g, e0):
                """one 2x row-tiled K=9 matmul pair of the hT chunk:
                hT = relu(W1.T posT + b1), stored fp8."""
                hT = hT_tiles[ci]
                e_lo = ci * CH_EDGES
                pha = psp.tile([P, 512], f32, tag="ps")
                phb = psp.tile([P, 512], f32, tag="ps")
                t0, t1 = 2 * g, 2 * g + 1
                nc.tensor.matmul(
                    out=pha[:],
                    lhsT=w1_sb[0:9, t0 * P : (t0 + 1) * P],
                    rhs=poseT_sb[0:9, e_lo + e0 : e_lo + e0 + 512],
                    start=True,
                    stop=True,
                    tile_position=(0, 0),
                )
                nc.tensor.matmul(
                    out=phb[:],
                    lhsT=w1_sb[32:41, t1 * P : (t1 + 1) * P],
                    rhs=poseT_sb[32:41, e_lo + e0 : e_lo + e0 + 512],
                    start=True,
                    stop=True,
                    tile_position=(32, 0),
                )
                for t, ph in ((t0, pha), (t1, phb)):
                    dstv = hT[:, t * CH_EDGES + e0 : t * CH_EDGES + e0 + 512]
                    # alternate PSUM evacuation between ACT and DVE
                    evac_ctr[0] += 1
                    if evac_ctr[0] % 3 != 2:
                        nc.scalar.activation(
                            dstv,
                            ph[:],
                            mybir.ActivationFunctionType.Relu,
                            bias=b1_sb[:, t : t + 1],
                            scale=1.0,
                        )
                    else:
                        nc.vector.tensor_scalar(
                            out=dstv,
                            in0=ph[:],
                            scalar1=b1_sb[:, t : t + 1],
                            scalar2=0.0,
                            op0=mybir.AluOpType.add,
                            op1=mybir.AluOpType.max,
                        )

            def pump_h(n):
                for _ in range(min(n, len(h_queue))):
                    emit_h_iter(*h_queue.pop(0))

            def flush_h(ci):
                """emit all pending h work for chunks <= ci"""
                queue_h(ci)
                while h_queue and h_queue[0][0] <= ci:
                    emit_h_iter(*h_queue.pop(0))

            def emit_gb(b, hT, ci):
                """gamma/beta for one 128-edge block -> bf16 [128, 2CS] half."""
                bi = b - ci * BPC
                pe_ps = psp.tile([P, 512], f32, tag="ps")
                if ET_MODE == "dr":
                    hT3 = hT.rearrange("p (t e) -> p t e", t=KT)
                    for t2 in range(KT // 2):
                        nc.tensor.matmul(
                            out=pe_ps[:, : 2 * CS],
                            lhsT=hT3[:, 2 * t2 : 2 * t2 + 2, bi * P : (bi + 1) * P],
                            rhs=w23[:, 2 * t2 : 2 * t2 + 2, :],
                            start=(t2 == 0),
                            stop=(t2 == KT // 2 - 1 and meta["b2_allzero"]),
                            perf_mode=mybir.MatmulPerfMode.DoubleRow,
                        )
                else:  # nodr: plain fp8, FWL weight loads
                    for t in range(KT):
                        nc.tensor.matmul(
                            out=pe_ps[:, : 2 * CS],
                            lhsT=hT[
                                :, t * CH_EDGES + bi * P : t * CH_EDGES + (bi + 1) * P
                            ],
                            rhs=w23[:, t, :],
                            start=(t == 0),
                            stop=(t == KT - 1 and meta["b2_allzero"]),
                        )
                if not meta["b2_allzero"]:
                    nc.tensor.matmul(
                        out=pe_ps[:, : 2 * CS],
                        lhsT=ones_sb[:1, :P],
                        rhs=b2_sb[:1, :],
                        start=False,
                        stop=True,
                    )
                return pe_ps

            # ---- main pipeline over pairs ----
            psw = None

            for pi_ in range(NP):
                ba, bb = pair_blk[pi_]
                w = pair_win[pi_]
                ci_need = bb // BPC
                flush_h(ci_need)  # h this pair depends on: emit now
                queue_h(ci_need + 1)  # next chunk's h: trickle between pairs

                # ---- gather X for the pair: [128, 2F]
                X2 = xp.tile([P, 2 * F], bf16, tag="xg")
                for j, b in enumerate((ba, bb)):
                    nc.gpsimd.indirect_dma_start(
                        out=X2[:, j * F : (j + 1) * F],
                        out_offset=None,
                        in_=img_d[:],
                        in_offset=bass.IndirectOffsetOnAxis(
                            ap=idx_sb[:, b : b + 1], axis=0
                        ),
                    )

                # ---- gamma (bf16, feeds DVE) and beta (m_dt, direct into m2's
                # 6th scatter segment) for both blocks
                gb2 = gbp.tile([P, 2 * CS], bf16, tag="gb")
                m2 = mp.tile([P, 2 * FS], m_dt, tag="mm")
                for j, b in enumerate((ba, bb)):
                    pe_ps = emit_gb(b, hT_tiles[b // BPC], b // BPC)
                    nc.scalar.activation(
                        gb2[:, j * CS : (j + 1) * CS],
                        pe_ps[:, :CS],
                        mybir.ActivationFunctionType.Sigmoid,
                    )
                    nc.scalar.activation(
                        m2[:, j * FS + F : (j + 1) * FS],
                        pe_ps[:, CS : 2 * CS],
                        mybir.ActivationFunctionType.Sigmoid,
                    )
                    pump_h(1)  # keep PE fed while sigmoid evacuates pe_ps

                # ---- m = gamma (bcast over hw) * X, per block
                for j in range(2):
                    g_b = (
                        gb2[:, j * CS : (j + 1) * CS]
                        .rearrange("p (o c) -> p o c", o=1)
                        .to_broadcast([P, HW, CS])
                    )
                    nc.vector.tensor_tensor(
                        out=m2[:, j * FS : j * FS + F].rearrange(
                            "p (o c) -> p o c", o=HW
                        ),
                        in0=X2[:, j * F : (j + 1) * F].rearrange(
                            "p (o c) -> p o c", o=HW
                        ),
                        in1=g_b,
                        op=mybir.AluOpType.mult,
                    )

                # ---- scatter matmuls into the window PSUM
                first = first_pair[w] == pi_
                last = last_pair[w] == pi_
                if first:
                    psw = pwp.tile([P, FS], f32, tag="pw")
                psw_l = psw

                if SC_MODE == "dr":
                    oh2 = oh_sb[:, pi_ * 2 * P : (pi_ + 1) * 2 * P].rearrange(
                        "p (j n) -> p j n", j=2
                    )
                    m3 = m2.rearrange("p (j f) -> p j f", j=2)
                    for s, width in seg_cols:
                        nc.tensor.matmul(
                            out=psw_l[:, s : s + width],
                            lhsT=oh2,
                            rhs=m3[:, :, s : s + width],
                            start=first,
                            stop=last,
                            perf_mode=mybir.MatmulPerfMode.DoubleRow,
                            skip_group_check=True,
                        )
                else:
                    for j, b in enumerate((ba, bb)):
                        oht = oh_sb[:, b * P : (b + 1) * P]
                        for s, width in seg_cols:
                            nc.tensor.matmul(
                                out=psw_l[:, s : s + width],
                                lhsT=oht,
                                rhs=m2[:, j * FS + s : j * FS + s + width],
                                start=first and j == 0,
                                stop=last and j == 1,
                                skip_group_check=True,
                            )

                pump_h(1)

                if last:
                    # ---- evacuate window: out = psw*recip + (beta_seg*recip)
                    # split by column halves across ACT and DVE so the PSUM
                    # window frees ~2x sooner (it gates the next window)
                    bs = outp.tile([P, CS], bf16, tag="bs")
                    nc.scalar.activation(
                        bs[:],
                        psw_l[:, F:FS],
                        mybir.ActivationFunctionType.Copy,
                        scale=recip_sb[:, w : w + 1],
                    )
                    HF = F // 2
                    HO = HW // 2
                    bs_b = bs.rearrange("p (o c) -> p o c", o=1)
                    of = outp.tile([P, F], bf16, tag="of")
                    of3 = of.rearrange("p (o c) -> p o c", o=HW)
                    psw3 = psw_l[:, :F].rearrange("p (o c) -> p o c", o=HW)
                    om = outp.tile([P, HF], bf16, tag="om")
                    nc.scalar.activation(
                        om[:],
                        psw_l[:, :HF],
                        mybir.ActivationFunctionType.Copy,
                        scale=recip_sb[:, w : w + 1],
                    )
                    nc.vector.scalar_tensor_tensor(
                        out=of3[:, HO:, :],
                        in0=psw3[:, HO:, :],
                        scalar=recip_sb[:, w : w + 1],
                        in1=bs_b.to_broadcast([P, HO, CS]),
                        op0=mybir.AluOpType.mult,
                        op1=mybir.AluOpType.add,
                    )
                    nc.vector.tensor_tensor(
                        out=of3[:, :HO, :],
                        in0=om.rearrange("p (o c) -> p o c", o=HO),
                        in1=bs_b.to_broadcast([P, HO, CS]),
                        op=mybir.AluOpType.add,
                    )
                    nc.sync.dma_start(out=out_d[w * P : (w + 1) * P, :], in_=of[:])

    _split_excess_waits(nc)
    return nc


def _run(inputs, trace=False, trace_kwargs=None):
    pose = np.asarray(inputs["pose"], np.float32)
    image = np.asarray(inputs["image"], np.float32)
    W1 = np.asarray(inputs["W1"], np.float32)
    b1 = np.asarray(inputs["b1"], np.float32)
    W2 = np.asarray(inputs["W2"], np.float32)
    b2 = np.asarray(inputs["b2"], np.float32)
    src = np.asarray(inputs["src"])
    dst = np.asarray(inputs["dst"])

    in_maps, meta = _host_prep(pose, image, W1, b1, W2, b2, src, dst)
    nc = _build(meta)
    kw = {}
    if trace:
        kw = dict(trace=True, trace_kwargs=trace_kwargs or {})
    res = run_bass_kernel_spmd(nc, in_maps, core_ids=list(range(N_CORES)), **kw)
    Nn, C, HW, CS = meta["Nn"], meta["C"], meta["HW"], meta["CS"]
    H = int(np.sqrt(HW))
    out = np.empty((Nn, C, H, HW // H), np.float32)
    for j in range(N_CORES):
        oc = np.asarray(res.results[j]["out"]).astype(np.float32)
        out[:, j * CS : (j + 1) * CS] = (
            oc.reshape(Nn, HW, CS).transpose(0, 2, 1).reshape(Nn, CS, H, HW // H)
        )
    return out, res


def kernel(**inputs) -> np.ndarray:
    out, _ = _run(inputs)
    return out


# revision 14
# speedup vs baseline: 1.2756x; 1.0778x over previous
"""GCN message-passing kernel for 8 Trainium2 NeuronCores.

Strategy: shard CHANNELS across the 8 cores (C=1280 -> 160 ch/core). Each core
computes the full output for its channel slice with zero collectives:
  - edge-encoder MLP: gamma/beta only for this core's 160 channels (W2 column
    shard), with the full h = relu(pose @ W1 + b1) recomputed per core on PE
    using 2x row-tiling (K=9 fits a 32-row group; two k-tiles run concurrently).
  - edges are host-sorted by dst and padded per 128-node dst window; the
    scatter-sum is a one-hot matmul on PE accumulating into PSUM per window.
    Blocks are processed in PAIRS with fp8 DoubleRow (K=256) so the one-hot
    weight load amortizes over 6 matmuls and the m stream halves.
  - the per-edge gather image[src] is an indirect DMA from an HBM-resident
    per-core image slice laid out [node, hw, ch] in bf16.
  - mean = PSUM evacuation with per-partition scale 1/cnt (0 for empty nodes).
"""

import sys

sys.path.insert(0, "/opt/trn_rl_repo")

import numpy as np
import ml_dtypes

import concourse.bass as bass
import concourse.mybir as mybir
from concourse.tile import TileContext
from concourse.bass_utils import run_bass_kernel_spmd

BF16 = ml_dtypes.bfloat16
FP8 = ml_dtypes.float8_e4m3
P = 128
N_CORES = 8
CH_EDGES = 1024  # edges per h-chunk
BPC = CH_EDGES // P  # blocks per chunk

ET_MODE = "dr"  # eT matmul: "nodr" (fp8+FWL) | "dr" (DoubleRow)
SC_MODE = "bf16"  # scatter: "dr" (fp8 DoubleRow pairs) | "bf16"


def _split_excess_waits(nc):
    """This walrus build only encodes 1 sem-wait per instruction; hoist extra
    waits onto same-engine NoOps placed just before (engines run in order)."""
    for bb in nc.main_func.blocks:
        new_insts = []
        for ins in bb.instructions:
            si = ins.sync_info
            limit = 1
            if si is not None and si.on_wait and len(si.on_wait) > limit:
                waits = list(si.on_wait)
                extra, keep = waits[:-limit], waits[-limit:]
                for k, w in enumerate(extra):
                    nop = mybir.InstNoOp(name=f"{ins.name}-ws-{k}", ins=[], outs=[])
                    nop.engine = ins.engine
                    nop.sync_info = mybir.SyncInfo(on_wait=[w], on_update=[])
                    new_insts.append(nop)
                si.on_wait = keep
            new_insts.append(ins)
        bb.instructions[:] = new_insts


def _host_prep(pose, image, W1, b1, W2, b2, src, dst):
    """Sort/pad edges by dst window, build per-core shards and onehot pairs."""
    E = pose.shape[0]
    Nn, C, H, Wsp = image.shape
    HW = H * Wsp
    CS = C // N_CORES
    F = CS * HW
    n_win = Nn // P

    src = np.asarray(src).astype(np.int64)
    dst = np.asarray(dst).astype(np.int64)

    order = np.argsort(dst, kind="stable")
    blk_edge = []  # [B, 128] edge id, -1 = pad
    blk_win = []
    for w in range(n_win):
        sel = order[(dst[order] >= w * P) & (dst[order] < (w + 1) * P)]
        nb = max(1, -(-len(sel) // P))
        if nb % 2:  # even block count per window -> clean DR pairs
            nb += 1
        for b in range(nb):
            seg = sel[b * P : (b + 1) * P]
            row = np.full(P, -1, np.int64)
            row[: len(seg)] = seg
            blk_edge.append(row)
            blk_win.append(w)
    blk_edge = np.stack(blk_edge)  # [B, 128]
    B = len(blk_edge)

    valid = blk_edge >= 0
    eids = np.where(valid, blk_edge, 0)

    # gather src per edge slot (pad -> node 0)
    blk_src = np.where(valid, src[eids], 0).astype(np.int32)  # [B,128]
    # onehot: [B, 128 edge, 128 local-node], zero row for pads
    loc = (np.where(valid, dst[eids], 0) - np.asarray(blk_win)[:, None] * P).astype(
        np.int64
    )
    oh = np.zeros((B, P, P), np.float32)
    bi, pi = np.nonzero(valid)
    oh[bi, pi, loc[bi, pi]] = 1.0

    # pairs (within window; B even per window by construction)
    pair_blk = []  # [NP, 2]
    pair_win = []
    b = 0
    while b < B:
        assert blk_win[b] == blk_win[b + 1]
        pair_blk.append((b, b + 1))
        pair_win.append(blk_win[b])
        b += 2
    NP = len(pair_blk)
    first_pair = {}
    last_pair = {}
    for pi_, w in enumerate(pair_win):
        first_pair.setdefault(w, pi_)
        last_pair[w] = pi_

    # edge chunks for h: pad edge count to CH_EDGES multiple
    Ep = -(-B * P // CH_EDGES) * CH_EDGES
    NCH = Ep // CH_EDGES

    # poseT padded: [9, Ep]
    pose_pad = np.zeros((Ep, 9), np.float32)
    pose_pad[: B * P] = np.where(valid.reshape(-1, 1), pose[eids.reshape(-1)], 0.0)
    poseT = np.ascontiguousarray(pose_pad.T.astype(BF16))  # [9, Ep]

    # oh, laid out for the scatter mode
    if SC_MODE == "dr":
        # per pair: [128 slot, 2*128]: [:, j*128+n] = oh[blk_j][slot, n], fp8
        oh_pairs = np.zeros((P, NP * 2 * P), np.float32)
        for pi_, (ba, bb) in enumerate(pair_blk):
            oh_pairs[:, pi_ * 2 * P : pi_ * 2 * P + P] = oh[ba]
            oh_pairs[:, pi_ * 2 * P + P : (pi_ + 1) * 2 * P] = oh[bb]
        oh_host = oh_pairs.astype(FP8)  # [128, NP*256]
    else:
        oh_host = (
            oh.transpose(1, 0, 2).reshape(P, B * P).astype(BF16)
        )  # [128, B*128]

    b2_allzero = not np.any(b2)
    cnt = np.bincount(dst, minlength=Nn).astype(np.float32)
    recip = np.where(cnt > 0, 1.0 / np.maximum(cnt, 1.0), 0.0).astype(np.float32)
    recip_t = np.ascontiguousarray(recip.reshape(n_win, P).T)  # [P, n_win]

    KT = C // P
    b1t = np.ascontiguousarray(b1.astype(np.float32).reshape(KT, P).T)  # [P, KT]
    idx_t = np.ascontiguousarray(blk_src.T)  # [P, B]

    shared = dict(
        poseT=poseT,
        w1=W1.astype(BF16),
        b1t=b1t,
        idx=idx_t,
        oh=oh_host,
        recip=recip_t,
    )
    in_maps = []
    for j in range(N_CORES):
        c0 = j * CS
        cols_g = [2 * (c0 + i) for i in range(CS)]
        cols_b = [2 * (c0 + i) + 1 for i in range(CS)]
        cols = cols_g + cols_b
        w2f8 = W2[:, cols].astype(FP8)  # [C, 2*CS]
        b2row = b2[cols].reshape(1, -1).astype(BF16)  # [1, 2*CS]
        img = (
            image[:, c0 : c0 + CS]
            .transpose(0, 2, 3, 1)
            .reshape(Nn, F)
            .astype(BF16)
        )  # [Nn, F] layout [n, hw, c]
        in_maps.append(dict(shared, w2f8=w2f8, b2row=b2row, image=img))

    meta = dict(
        E=E, Nn=Nn, C=C, HW=HW, CS=CS, F=F, n_win=n_win, B=B, Ep=Ep, KT=KT,
        NP=NP, NCH=NCH, pair_blk=pair_blk, pair_win=pair_win,
        first_pair=first_pair, last_pair=last_pair, b2_allzero=b2_allzero,
    )
    return in_maps, meta


def _build(meta):
    Nn, CS, F, HW = meta["Nn"], meta["CS"], meta["F"], meta["HW"]
    n_win, B, Ep, KT = meta["n_win"], meta["B"], meta["Ep"], meta["KT"]
    C, NP, NCH = meta["C"], meta["NP"], meta["NCH"]
    pair_blk, pair_win = meta["pair_blk"], meta["pair_win"]
    first_pair, last_pair = meta["first_pair"], meta["last_pair"]
    f32 = mybir.dt.float32
    bf16 = mybir.dt.bfloat16
    fp8 = mybir.dt.float8e4
    i32 = mybir.dt.int32
    FS = F + CS  # scatter width: gamma*x features + beta column block
    seg_cols = [(s, min(512, FS - s)) for s in range(0, FS, 512)]
    m_dt = fp8 if SC_MODE == "dr" else bf16
    oh_dt = fp8 if SC_MODE == "dr" else bf16
    oh_w = NP * 2 * P if SC_MODE == "dr" else B * P

    nc = bass.Bass()
    poseT_d = nc.declare_dram_parameter("poseT", [9, Ep], bf16, isOutput=False)
    w1_d = nc.declare_dram_parameter("w1", [9, C], bf16, isOutput=False)
    b1t_d = nc.declare_dram_parameter("b1t", [P, KT], f32, isOutput=False)
    w2f8_d = nc.declare_dram_parameter("w2f8", [C, 2 * CS], fp8, isOutput=False)
    b2_d = nc.declare_dram_parameter("b2row", [1, 2 * CS], bf16, isOutput=False)
    img_d = nc.declare_dram_parameter("image", [Nn, F], bf16, isOutput=False)
    idx_d = nc.declare_dram_parameter("idx", [P, B], i32, isOutput=False)
    oh_d = nc.declare_dram_parameter("oh", [P, oh_w], oh_dt, isOutput=False)
    recip_d = nc.declare_dram_parameter("recip", [P, n_win], f32, isOutput=False)
    out_d = nc.declare_dram_parameter("out", [Nn, F], bf16, isOutput=True)

    with TileContext(nc) as tc:
        with (
            tc.tile_pool(name="const", bufs=1) as constp,
            tc.tile_pool(name="ht", bufs=3) as htp,
            tc.tile_pool(name="gb", bufs=6) as gbp,
            tc.tile_pool(name="xg", bufs=6) as xp,
            tc.tile_pool(name="mm", bufs=4) as mp,
            tc.tile_pool(name="outp", bufs=2) as outp,
            tc.tile_pool(name="pw", bufs=1, space="PSUM") as pwp,
            tc.tile_pool(name="ps", bufs=2, space="PSUM") as psp,
        ):
            # ---- preload constants (first chunk's pose slice first, so the
            # PE can start immediately; the rest streams behind it) ----
            w1_sb = constp.tile([41, C], bf16)
            nc.sync.dma_start(out=w1_sb[0:9, :], in_=w1_d[:])
            nc.sync.dma_start(out=w1_sb[32:41, :], in_=w1_d[:])
            poseT_sb = constp.tile([41, Ep], bf16)
            nc.scalar.dma_start(
                out=poseT_sb[0:9, :CH_EDGES], in_=poseT_d[:, :CH_EDGES]
            )
            nc.scalar.dma_start(
                out=poseT_sb[32:41, :CH_EDGES], in_=poseT_d[:, :CH_EDGES]
            )
            nc.sync.dma_start(out=poseT_sb[0:9, CH_EDGES:], in_=poseT_d[:, CH_EDGES:])
            nc.sync.dma_start(
                out=poseT_sb[32:41, CH_EDGES:], in_=poseT_d[:, CH_EDGES:]
            )
            b1_sb = constp.tile([P, KT], f32)
            nc.sync.dma_start(out=b1_sb[:], in_=b1t_d[:])
            w2_sb = constp.tile([P, KT * 2 * CS], fp8)
            for t in range(KT):
                nc.sync.dma_start(
                    out=w2_sb[:, t * 2 * CS : (t + 1) * 2 * CS],
                    in_=w2f8_d[t * P : (t + 1) * P, :],
                )
            b2_sb = constp.tile([1, 2 * CS], bf16)
            if not meta["b2_allzero"]:
                nc.sync.dma_start(out=b2_sb[:], in_=b2_d[:])
            idx_sb = constp.tile([P, B], i32)
            nc.sync.dma_start(out=idx_sb[:], in_=idx_d[:])
            oh_sb = constp.tile([P, oh_w], oh_dt)
            nc.sync.dma_start(out=oh_sb[:], in_=oh_d[:])
            recip_sb = constp.tile([P, n_win], f32)
            nc.sync.dma_start(out=recip_sb[:], in_=recip_d[:])
            ones_sb = constp.tile([1, P], bf16)
            if not meta["b2_allzero"]:
                nc.gpsimd.memset(ones_sb[:], 1.0)

            w23 = w2_sb.rearrange("p (t c) -> p t c", t=KT)

            hT_tiles = {}  # chunk -> tile
            h_queue = []  # pending (ci, g, e0) h iterations, FIFO
            h_queued_ci = -1  # last chunk whose iterations were enqueued
            evac_ctr = [0]

            def queue_h(ci):
                nonlocal h_queued_ci
                while h_queued_ci < ci:
                    h_queued_ci += 1
                    if h_queued_ci >= NCH:
                        return
                    hT_tiles[h_queued_ci] = htp.tile(
                        [P, KT * CH_EDGES], fp8, tag="ht", name=f"hT{h_queued_ci}"
                    )
                    for g in range(KT // 2):
                        for e0 in range(0, CH_EDGES, 512):
                            h_queue.append((h_queued_ci, g, e0))

            def emit_h_iter(ci, g, e0):
                """one 2x row-tiled K=9 matmul pair of the hT chunk:
                hT = relu(W1.T posT + b1), stored fp8."""
                hT = hT_tiles[ci]
                e_lo = ci * CH_EDGES
                pha = psp.tile([P, 512], f32, tag="ps")
                phb = psp.tile([P, 512], f32, tag="ps")
                t0, t1 = 2 * g, 2 * g + 1
                nc.tensor.matmul(
                    out=pha[:],
                    lhsT=w1_sb[0:9, t0 * P : (t0 + 1) * P],
                    rhs=poseT_sb[0:9, e_lo + e0 : e_lo + e0 + 512],
                    start=True,
                    stop=True,
                    tile_position=(0, 0),
                )
                nc.tensor.matmul(
                    out=phb[:],
                    lhsT=w1_sb[32:41, t1 * P : (t1 + 1) * P],
                    rhs=poseT_sb[32:41, e_lo + e0 : e_lo + e0 + 512],
                    start=True,
                    stop=True,
                    tile_position=(32, 0),
                )
                for t, ph in ((t0, pha), (t1, phb)):
                    dstv = hT[:, t * CH_EDGES + e0 : t * CH_EDGES + e0 + 512]
                    # alternate PSUM evacuation between ACT and DVE
                    evac_ctr[0] += 1
                    if evac_ctr[0] % 3 != 2:
                        nc.scalar.activation(
                            dstv,
                            ph[:],
                            mybir.ActivationFunctionType.Relu,
                            bias=b1_sb[:, t : t + 1],
                            scale=1.0,
                        )
                    else:
                        nc.vector.tensor_scalar(
                            out=dstv,
                            in0=ph[:],
                            scalar1=b1_sb[:, t : t + 1],
                            scalar2=0.0,
                            op0=mybir.AluOpType.add,
                            op1=mybir.AluOpType.max,
                        )

            def pump_h(n):
                for _ in range(min(n, len(h_queue))):
                    emit_h_iter(*h_queue.pop(0))

            def flush_h(ci):
                """emit all pending h work for chunks <= ci"""
                queue_h(ci)
                while h_queue and h_queue[0][0] <= ci:
                    emit_h_iter(*h_queue.pop(0))

            def emit_gb(b, hT, ci):
                """gamma/beta for one 128-edge block -> bf16 [128, 2CS] half."""
                bi = b - ci * BPC
                pe_ps = psp.tile([P, 512], f32, tag="ps")
                if ET_MODE == "dr":
                    hT3 = hT.rearrange("p (t e) -> p t e", t=KT)
                    for t2 in range(KT // 2):
                        nc.tensor.matmul(
                            out=pe_ps[:, : 2 * CS],
                            lhsT=hT3[:, 2 * t2 : 2 * t2 + 2, bi * P : (bi + 1) * P],
                            rhs=w23[:, 2 * t2 : 2 * t2 + 2, :],
                            start=(t2 == 0),
                            stop=(t2 == KT // 2 - 1 and meta["b2_allzero"]),
                            perf_mode=mybir.MatmulPerfMode.DoubleRow,
                        )
                else:  # nodr: plain fp8, FWL weight loads
                    for t in range(KT):
                        nc.tensor.matmul(
                            out=pe_ps[:, : 2 * CS],
                            lhsT=hT[
                                :, t * CH_EDGES + bi * P : t * CH_EDGES + (bi + 1) * P
                            ],
                            rhs=w23[:, t, :],
                            start=(t == 0),
                            stop=(t == KT - 1 and meta["b2_allzero"]),
                        )
                if not meta["b2_allzero"]:
                    nc.tensor.matmul(
                        out=pe_ps[:, : 2 * CS],
                        lhsT=ones_sb[:1, :P],
                        rhs=b2_sb[:1, :],
                        start=False,
                        stop=True,
                    )
                return pe_ps

            # ---- main pipeline over pairs ----
            psw = None

            for pi_ in range(NP):
                ba, bb = pair_blk[pi_]
                w = pair_win[pi_]
                ci_need = bb // BPC
                flush_h(ci_need)  # h this pair depends on: emit now
                queue_h(ci_need + 1)  # next chunk's h: trickle between pairs

                # ---- gather X for the pair: [128, 2F]
                X2 = xp.tile([P, 2 * F], bf16, tag="xg")
                for j, b in enumerate((ba, bb)):
                    nc.gpsimd.indirect_dma_start(
                        out=X2[:, j * F : (j + 1) * F],
                        out_offset=None,
                        in_=img_d[:],
                        in_offset=bass.IndirectOffsetOnAxis(
                            ap=idx_sb[:, b : b + 1], axis=0
                        ),
                    )

                # ---- gamma (bf16, feeds DVE) and beta (m_dt, direct into m2's
                # 6th scatter segment) for both blocks
                gb2 = gbp.tile([P, 2 * CS], bf16, tag="gb")
                m2 = mp.tile([P, 2 * FS], m_dt, tag="mm")
                for j, b in enumerate((ba, bb)):
                    pe_ps = emit_gb(b, hT_tiles[b // BPC], b // BPC)
                    nc.scalar.activation(
                        gb2[:, j * CS : (j + 1) * CS],
                        pe_ps[:, :CS],
                        mybir.ActivationFunctionType.Sigmoid,
                    )
                    nc.scalar.activation(
                        m2[:, j * FS + F : (j + 1) * FS],
                        pe_ps[:, CS : 2 * CS],
                        mybir.ActivationFunctionType.Sigmoid,
                    )
                    pump_h(1)  # keep PE fed while sigmoid evacuates pe_ps

                # ---- m = gamma (bcast over hw) * X, per block
                for j in range(2):
                    g_b = (
                        gb2[:, j * CS : (j + 1) * CS]
                        .rearrange("p (o c) -> p o c", o=1)
                        .to_broadcast([P, HW, CS])
                    )
                    nc.vector.tensor_tensor(
                        out=m2[:, j * FS : j * FS + F].rearrange(
                            "p (o c) -> p o c", o=HW
                        ),
                        in0=X2[:, j * F : (j + 1) * F].rearrange(
                            "p (o c) -> p o c", o=HW
                        ),
                        in1=g_b,
                        op=mybir.AluOpType.mult,
                    )

                # ---- scatter matmuls into the window PSUM
                first = first_pair[w] == pi_
                last = last_pair[w] == pi_
                if first:
                    psw = pwp.tile([P, FS], f32, tag="pw")
                psw_l = psw

                if SC_MODE == "dr":
                    oh2 = oh_sb[:, pi_ * 2 * P : (pi_ + 1) * 2 * P].rearrange(
                        "p (j n) -> p j n", j=2
                    )
                    m3 = m2.rearrange("p (j f) -> p j f", j=2)
                    for s, width in seg_cols:
                        nc.tensor.matmul(
                            out=psw_l[:, s : s + width],
                            lhsT=oh2,
                            rhs=m3[:, :, s : s + width],
                            start=first,
                            stop=last,
                            perf_mode=mybir.MatmulPerfMode.DoubleRow,
                            skip_group_check=True,
                        )
                else:
                    for j, b in enumerate((ba, bb)):
                        oht = oh_sb[:, b * P : (b + 1) * P]
                        for s, width in seg_cols:
                            nc.tensor.matmul(
                                out=psw_l[:, s : s + width],
                                lhsT=oht,
                                rhs=m2[:, j * FS + s : j * FS + s + width],
                                start=first and j == 0,
                                stop=last and j == 1,
                                skip_group_check=True,
                            )

                pump_h(1)

                if last:
                    # ---- evacuate window: out = psw*recip + (beta_seg*recip)
                    # split by column halves across ACT and DVE so the PSUM
                    # window frees ~2x sooner (it gates the next window)
                    bs = outp.tile([P, CS], bf16, tag="bs")
                    nc.scalar.activation(
                        bs[:],
                        psw_l[:, F:FS],
                        mybir.ActivationFunctionType.Copy,
                        scale=recip_sb[:, w : w + 1],
                    )
                    HF = F // 2
                    HO = HW // 2
                    bs_b = bs.rearrange("p (o c) -> p o c", o=1)
                    of = outp.tile([P, F], bf16, tag="of")
                    of3 = of.rearrange("p (o c) -> p o c", o=HW)
                    psw3 = psw_l[:, :F].rearrange("p (o c) -> p o c", o=HW)
                    om = outp.tile([P, HF], bf16, tag="om")
                    nc.scalar.activation(
                        om[:],
                        psw_l[:, :HF],
                        mybir.ActivationFunctionType.Copy,
                        scale=recip_sb[:, w : w + 1],
                    )
                    nc.vector.scalar_tensor_tensor(
                        out=of3[:, HO:, :],
                        in0=psw3[:, HO:, :],
                        scalar=recip_sb[:, w : w + 1],
                        in1=bs_b.to_broadcast([P, HO, CS]),
                        op0=mybir.AluOpType.mult,
                        op1=mybir.AluOpType.add,
                    )
                    nc.vector.tensor_tensor(
                        out=of3[:, :HO, :],
                        in0=om.rearrange("p (o c) -> p o c", o=HO),
                        in1=bs_b.to_broadcast([P, HO, CS]),
                        op=mybir.AluOpType.add,
                    )
                    nc.sync.dma_start(out=out_d[w * P : (w + 1) * P, :], in_=of[:])

    _split_excess_waits(nc)
    return nc


def _run(inputs, trace=False, trace_kwargs=None):
    pose = np.asarray(inputs["pose"], np.float32)
    image = np.asarray(inputs["image"], np.float32)
    W1 = np.asarray(inputs["W1"], np.float32)
    b1 = np.asarray(inputs["b1"], np.float32)
    W2 = np.asarray(inputs["W2"], np.float32)
    b2 = np.asarray(inputs["b2"], np.float32)
    src = np.asarray(inputs["src"])
    dst = np.asarray(inputs["dst"])

    in_maps, meta = _host_prep(pose, image, W1, b1, W2, b2, src, dst)
    nc = _build(meta)
    kw = {}
    if trace:
        kw = dict(trace=True, trace_kwargs=trace_kwargs or {})
    res = run_bass_kernel_spmd(nc, in_maps, core_ids=list(range(N_CORES)), **kw)
    Nn, C, HW, CS = meta["Nn"], meta["C"], meta["HW"], meta["CS"]
    H = int(np.sqrt(HW))
    out = np.empty((Nn, C, H, HW // H), np.float32)
    for j in range(N_CORES):
        oc = np.asarray(res.results[j]["out"]).astype(np.float32)
        out[:, j * CS : (j + 1) * CS] = (
            oc.reshape(Nn, HW, CS).transpose(0, 2, 1).reshape(Nn, CS, H, HW // H)
        )
    return out, res


def kernel(**inputs) -> np.ndarray:
    out, _ = _run(inputs)
    return out
